# revision 20
# baseline (speedup 1.0000x reference)
"""Trainium2 Bass kernel for nn_CLinear (group-quantized linear layer).

Computes out = x @ dequant(qweight).T + bias where
  x:       [4, 2048, 4096] f32
  qweight: [11008, 16, 256] int8 (group-quantized, G=256)
  scale:   [11008, 16, 1]   f32  (w = qweight / scale)
  bias:    [11008]          f32
  out:     [4, 2048, 11008] f32

Sharding: column-parallel (tensor-parallel over out_features) across 8
NeuronCores.  OUT is padded 11008 -> 11264 = 8 * 1408 so every core gets
11 full 128-row tiles.  x is replicated to every core.

All activation/weight preprocessing happens on the host: x is cast to
bf16 and laid out as K-major lhsT tiles xt[m, p, u, t] = x[128m+t, 128u+p];
the weight shard is dequantized to bf16 and transposed to
wt[p, u, o] = w[o, 128u+p].  The device kernel is then a pure streaming
GEMM: resident weights + bias, stream x tiles in, 32 accumulating bf16
matmuls per (token-tile, out-block) into PSUM f32 (k-chunk-outer order so
the PE only ever waits for the earliest unarrived weight chunk), DVE adds
bias on PSUM->SBUF evict, DMA the f32 result out.
"""

import numpy as np

import concourse.bass as bass
import concourse.mybir as mybir
import concourse.tile as tile
from concourse import bacc
from concourse.bass_utils import run_bass_kernel_spmd

P = 128
B, S, IN, OUT, G = 4, 2048, 4096, 11008, 256
NCORES = 8
T = B * S                      # 8192 tokens
OUT_PAD = OUT                  # 11008 = 8 * 1376, no padding needed
OUT_SH = OUT_PAD // NCORES     # 1376 out features per core (512+512+352)
NG = IN // G                   # 16 quant groups per row
KT = IN // P                   # 32 k-tiles
MT = T // P                    # 64 token tiles
UCH = 4                        # k-tiles per resident weight chunk
XSPLIT = 1                     # sub-DMAs per x tile
XU = KT // XSPLIT              # k-tiles per x sub-tile
F32 = mybir.dt.float32
BF16 = mybir.dt.bfloat16

NBLK = [(0, 512), (512, 512), (1024, OUT_SH - 1024)]


def emit_kernel(tc, nc, xt_d, wt_d, bb_d, y_d):
    """Per-core kernel IR.

    xt_d: [MT, P, KT, P] bf16  (host-transposed lhsT tiles, replicated)
    wt_d: [P, KT, OUT_SH] bf16 (host-dequantized transposed weight shard)
    bb_d: [P, OUT_SH]     f32  (bias shard broadcast along partitions)
    y_d:  [T, OUT_SH]     f32  (output shard)
    """
    from contextlib import ExitStack
    ctx = ExitStack()
    const = ctx.enter_context(tc.tile_pool(name="const", bufs=1))
    wtp = ctx.enter_context(tc.tile_pool(name="wt", bufs=1))
    xp = ctx.enter_context(tc.tile_pool(name="x", bufs=5))
    outp = ctx.enter_context(tc.tile_pool(name="out", bufs=3))
    psp = ctx.enter_context(tc.tile_pool(name="psum", bufs=2, space="PSUM"))
    psx = ctx.enter_context(tc.tile_pool(name="psumx", bufs=1, space="PSUM"))

    def produce(m, splits=None):
        """Load x tile m; returns [(u0, u1, ap), ...] sub-pieces."""
        subs = []
        for (u0, u1) in (splits or [(0, KT)]):
            xt = xp.tile([P, u1 - u0, P], BF16, name=f"xt{u0}_{u1}")
            nc.sync.dma_start(xt[:], xt_d[m, :, u0:u1, :])
            subs.append((u0, u1, xt))
        return subs

    HALF = [(0, KT // 2), (KT // 2, KT)]
    # First three x tiles in half-tile pieces so early ladder stages start
    # as soon as the first halves land.
    xts = {m: produce(m, HALF) for m in range(min(3, MT))}

    # Resident weight pieces: fine-grained at the front (so the PE starts
    # as early as possible), coarse after.  Split over scalar+gpsimd in
    # consumption order; sync stays clear for x.
    WPIECES = [  # (u0, u1, queue)
        (0, 2, nc.scalar), (4, 6, nc.gpsimd),
        (2, 4, nc.scalar), (6, 8, nc.gpsimd),
        (8, 12, nc.scalar), (12, 16, nc.gpsimd),
        (16, 20, nc.scalar), (20, 24, nc.gpsimd),
        (24, 28, nc.scalar), (28, 32, nc.gpsimd),
    ]
    wts = []          # list of (u0, u1, ap)
    umap = {}         # u -> (piece_idx, offset)
    for (u0, u1, eng) in WPIECES:
        wtt = wtp.tile([P, u1 - u0, OUT_SH], BF16, name=f"wt{u0}_{u1}")
        eng.dma_start(wtt[:], wt_d[:, u0:u1, :])
        for u in range(u0, u1):
            umap[u] = (len(wts), u - u0)
        wts.append((u0, u1, wtt))
    # Bias last on gpsimd: first needed at the first evict.
    biasb = const.tile([P, OUT_SH], F32)
    nc.gpsimd.dma_start(biasb[:], bb_d[:, :])

    sq = [nc.gpsimd, nc.scalar, nc.sync]

    def evict(m, nb, n0, sz, ps, qi=None):
        t0 = m * P
        ot = outp.tile([P, 512], F32, name=f"ot{nb}")
        nc.vector.tensor_tensor(
            ot[:, :sz], ps, biasb[:, n0:n0 + sz], mybir.AluOpType.add
        )
        sq[(m if qi is None else qi) % 3].dma_start(
            y_d[t0:t0 + P, n0:n0 + sz], ot[:, :sz])

    def alloc_ps():
        return [psp.tile([P, 512], F32, name=f"ps{nb}")[:, :sz]
                for nb, (n0, sz) in enumerate(NBLK)]

    def mm(pss, xtf, u, nb, n0, sz, started=None):
        for (xu0, xu1, xap) in xtf:
            if xu0 <= u < xu1:
                break
        wi, wo = umap[u]
        key = (id(pss), nb)
        first = started is None or key not in started
        if started is not None:
            started.add(key)
        nc.tensor.matmul(
            pss[nb],
            xap[:, u - xu0, :],
            wts[wi][2][:, wo, n0:n0 + sz],
            start=(u == 0) if started is None else first,
            stop=(u == KT - 1),
        )

    # Startup ladder: interleave the first 2 2/3 token tiles across all 8
    # PSUM banks, with PE stages emitted in the exact order the x/weight
    # pieces arrive on their queues (PSUM accumulation order over u is
    # free).  Tile 2's last block is deferred to the main loop (bank
    # budget).
    for m in range(3, min(5, MT)):
        xts[m] = produce(m)
    nextp = min(5, MT)
    spss = [alloc_ps() for _ in range(2)]
    sps2 = [psx.tile([P, 512], F32, name=f"xps{nb}")[:, :sz]
            for nb, (n0, sz) in enumerate(NBLK[:2])]
    started = set()
    STAGES = [
        (0, 0, 2), (0, 4, 6), (0, 2, 4), (0, 6, 8),
        (1, 0, 8),
        (0, 8, 16), (1, 8, 16),
        (2, 0, 16),
        (0, 16, 24), (1, 16, 24),
        (2, 16, 24),
        (0, 24, 32), (1, 24, 32),
        (2, 24, 32),
    ]
    for (ti, ulo, uhi) in STAGES:
        for u in range(ulo, uhi):
            if ti < 2:
                for nb, (n0, sz) in enumerate(NBLK):
                    mm(spss[ti], xts[ti], u, nb, n0, sz, started)
            else:
                for nb, (n0, sz) in enumerate(NBLK[:2]):
                    mm(sps2, xts[2], u, nb, n0, sz, started)
    for i in range(2):
        xts.pop(i)
        for nb, (n0, sz) in enumerate(NBLK):
            evict(i, nb, n0, sz, spss[i][nb])

    for m in range(2, MT):
        if nextp < MT:
            xts[nextp] = produce(nextp)
            nextp += 1
        xtf = xts.pop(m)
        if m == 2:
            # Finish tile 2: only its last block remains, then evict all.
            nb2, (n20, s2z) = 2, NBLK[2]
            ps2 = psp.tile([P, 512], F32, name="ps2")[:, :s2z]
            for u in range(KT):
                mm([None, None, ps2], xtf, u, nb2, n20, s2z)
            for nb, (n0, sz) in enumerate(NBLK[:2]):
                evict(m, nb, n0, sz, sps2[nb])
            evict(m, nb2, n20, s2z, ps2)
            continue
        pss = alloc_ps()
        if m == MT - 1:
            # Last tile: block-sequential so each block's evict+store
            # overlaps the remaining blocks' matmuls, and stores rotated
            # across queues so the final drain flushes in parallel.
            for nb, (n0, sz) in enumerate(NBLK):
                for u in range(KT):
                    mm(pss, xtf, u, nb, n0, sz)
                evict(m, nb, n0, sz, pss[nb], qi=m + nb)
        else:
            # k-chunk-outer so weight chunks are consumed in arrival order.
            for u in range(KT):
                for nb, (n0, sz) in enumerate(NBLK):
                    mm(pss, xtf, u, nb, n0, sz)
            for nb, (n0, sz) in enumerate(NBLK):
                evict(m, nb, n0, sz, pss[nb])

    ctx.close()


def build_nc(debug=False):
    nc = bacc.Bacc(
        "TRN2",
        target_bir_lowering=False,
        debug=debug,
        num_devices=NCORES,
        enable_asserts=debug,
    )
    xt_d = nc.dram_tensor("xt", [MT, P, KT, P], BF16, kind="ExternalInput").ap()
    wt_d = nc.dram_tensor("wt", [P, KT, OUT_SH], BF16, kind="ExternalInput").ap()
    bb_d = nc.dram_tensor("biasb", [P, OUT_SH], F32, kind="ExternalInput").ap()
    y_d = nc.dram_tensor("y", [T, OUT_SH], F32, kind="ExternalOutput").ap()
    with tile.TileContext(nc) as tc:
        emit_kernel(tc, nc, xt_d, wt_d, bb_d, y_d)
    nc.compile()
    return nc


_NC_CACHE = {}


def _get_nc():
    if "nc" not in _NC_CACHE:
        _NC_CACHE["nc"] = build_nc()
    return _NC_CACHE["nc"]


def prep_inputs(x, qweight, scale, bias):
    """Host-side prep. Returns in_maps for run_bass_kernel_spmd."""
    import ml_dtypes
    x = np.asarray(x)
    qw = np.asarray(qweight)
    sc = np.asarray(scale, dtype=np.float32)
    b = np.asarray(bias, dtype=np.float32)

    # xt[m, p, u, t] = x[128m + t, 128u + p], bf16
    x2 = x.reshape(T, IN).astype(ml_dtypes.bfloat16)
    xt = np.ascontiguousarray(
        x2.reshape(MT, P, KT, P).transpose(0, 3, 2, 1))

    # Dequantize exactly as the reference does (q / scale, f32), then bf16.
    qw2 = qw.reshape(OUT, NG, G)
    w = (qw2.astype(np.float32) / sc.reshape(OUT, NG, 1)).reshape(OUT, IN)
    w_p = np.zeros((OUT_PAD, IN), dtype=ml_dtypes.bfloat16)
    w_p[:OUT] = w.astype(ml_dtypes.bfloat16)
    b_p = np.zeros(OUT_PAD, dtype=np.float32)
    b_p[:OUT] = b

    in_maps = []
    for c in range(NCORES):
        sl = slice(c * OUT_SH, (c + 1) * OUT_SH)
        # wt[p, u, o] = w[o, 128u + p]
        wt = np.ascontiguousarray(
            w_p[sl].reshape(OUT_SH, KT, P).transpose(2, 1, 0))
        in_maps.append({
            "xt": xt,
            "wt": wt,
            "biasb": np.ascontiguousarray(
                np.broadcast_to(b_p[sl][None, :], (P, OUT_SH))
            ),
        })
    return in_maps


def run(x, qweight, scale, bias, trace=False):
    nc = _get_nc()
    in_maps = prep_inputs(x, qweight, scale, bias)
    res = run_bass_kernel_spmd(nc, in_maps, core_ids=list(range(NCORES)),
                               trace=trace)
    ys = [np.asarray(res.results[c]["y"]) for c in range(NCORES)]
    out = np.concatenate(ys, axis=1)[:, :OUT]
    return out.reshape(B, S, OUT).astype(np.float32, copy=False), res


def kernel(x, qweight, scale, bias):
    out, _ = run(x, qweight, scale, bias, trace=False)
    return out


# revision 22
# speedup vs baseline: 1.0392x; 1.0392x over previous
"""Trainium2 Bass kernel for nn_CLinear (group-quantized linear layer).

Computes out = x @ dequant(qweight).T + bias where
  x:       [4, 2048, 4096] f32
  qweight: [11008, 16, 256] int8 (group-quantized, G=256)
  scale:   [11008, 16, 1]   f32  (w = qweight / scale)
  bias:    [11008]          f32
  out:     [4, 2048, 11008] f32

Sharding: column-parallel (tensor-parallel over out_features) across 8
NeuronCores.  OUT is padded 11008 -> 11264 = 8 * 1408 so every core gets
11 full 128-row tiles.  x is replicated to every core.

All activation/weight preprocessing happens on the host: x is cast to
bf16 and laid out as K-major lhsT tiles xt[m, p, u, t] = x[128m+t, 128u+p];
the weight shard is dequantized to bf16 and transposed to
wt[p, u, o] = w[o, 128u+p].  The device kernel is then a pure streaming
GEMM: resident weights + bias, stream x tiles in, 32 accumulating bf16
matmuls per (token-tile, out-block) into PSUM f32 (k-chunk-outer order so
the PE only ever waits for the earliest unarrived weight chunk), DVE adds
bias on PSUM->SBUF evict, DMA the f32 result out.
"""

import numpy as np

import concourse.bass as bass
import concourse.mybir as mybir
import concourse.tile as tile
from concourse import bacc
from concourse.bass_utils import run_bass_kernel_spmd

P = 128
B, S, IN, OUT, G = 4, 2048, 4096, 11008, 256
NCORES = 8
T = B * S                      # 8192 tokens
OUT_PAD = OUT                  # 11008 = 8 * 1376, no padding needed
OUT_SH = OUT_PAD // NCORES     # 1376 out features per core (512+512+352)
NG = IN // G                   # 16 quant groups per row
KT = IN // P                   # 32 k-tiles
MT = T // P                    # 64 token tiles
UCH = 4                        # k-tiles per resident weight chunk
XSPLIT = 1                     # sub-DMAs per x tile
XU = KT // XSPLIT              # k-tiles per x sub-tile
F32 = mybir.dt.float32
BF16 = mybir.dt.bfloat16

NBLK = [(0, 512), (512, 512), (1024, OUT_SH - 1024)]


def emit_kernel(tc, nc, xt_d, wt_d, bb_d, y_d):
    """Per-core kernel IR.

    xt_d: [MT, P, KT, P] bf16  (host-transposed lhsT tiles, replicated)
    wt_d: [P, KT, OUT_SH] bf16 (host-dequantized transposed weight shard)
    bb_d: [P, OUT_SH]     f32  (bias shard broadcast along partitions)
    y_d:  [T, OUT_SH]     f32  (output shard)
    """
    from contextlib import ExitStack
    ctx = ExitStack()
    const = ctx.enter_context(tc.tile_pool(name="const", bufs=1))
    wtp = ctx.enter_context(tc.tile_pool(name="wt", bufs=1))
    xp = ctx.enter_context(tc.tile_pool(name="x", bufs=5))
    outp = ctx.enter_context(tc.tile_pool(name="out", bufs=3))
    psp = ctx.enter_context(tc.tile_pool(name="psum", bufs=2, space="PSUM"))
    psx = ctx.enter_context(tc.tile_pool(name="psumx", bufs=1, space="PSUM"))

    def produce(m, splits=None):
        """Load x tile m; returns [(u0, u1, ap), ...] sub-pieces."""
        subs = []
        for (u0, u1) in (splits or [(0, KT)]):
            xt = xp.tile([P, u1 - u0, P], BF16, name=f"xt{u0}_{u1}")
            nc.sync.dma_start(xt[:], xt_d[m, :, u0:u1, :])
            subs.append((u0, u1, xt))
        return subs

    xts = {m: produce(m) for m in range(min(3, MT))}

    # Resident weight pieces split over scalar+gpsimd in consumption
    # order (g7 on sync behind the first x tiles); sync otherwise clear
    # for x.
    WPIECES = [  # (u0, u1, queue)
        (0, 4, nc.scalar), (4, 8, nc.gpsimd),
        (8, 12, nc.scalar), (12, 16, nc.gpsimd),
        (16, 20, nc.scalar), (20, 24, nc.gpsimd),
        (24, 28, nc.scalar), (28, 32, nc.sync),
    ]
    wts = []          # list of (u0, u1, ap)
    umap = {}         # u -> (piece_idx, offset)
    for (u0, u1, eng) in WPIECES:
        wtt = wtp.tile([P, u1 - u0, OUT_SH], BF16, name=f"wt{u0}_{u1}")
        eng.dma_start(wtt[:], wt_d[:, u0:u1, :])
        for u in range(u0, u1):
            umap[u] = (len(wts), u - u0)
        wts.append((u0, u1, wtt))
    # Bias last on gpsimd: first needed at the first evict.
    biasb = const.tile([P, OUT_SH], F32)
    nc.gpsimd.dma_start(biasb[:], bb_d[:, :])

    sq = [nc.gpsimd, nc.scalar, nc.sync]

    def evict(m, nb, n0, sz, ps, qi=None):
        t0 = m * P
        ot = outp.tile([P, 512], F32, name=f"ot{nb}")
        nc.vector.tensor_tensor(
            ot[:, :sz], ps, biasb[:, n0:n0 + sz], mybir.AluOpType.add
        )
        sq[(m if qi is None else qi) % 3].dma_start(
            y_d[t0:t0 + P, n0:n0 + sz], ot[:, :sz])

    def alloc_ps():
        return [psp.tile([P, 512], F32, name=f"ps{nb}")[:, :sz]
                for nb, (n0, sz) in enumerate(NBLK)]

    def mm(pss, xtf, u, nb, n0, sz, started=None):
        for (xu0, xu1, xap) in xtf:
            if xu0 <= u < xu1:
                break
        wi, wo = umap[u]
        key = (id(pss), nb)
        first = started is None or key not in started
        if started is not None:
            started.add(key)
        nc.tensor.matmul(
            pss[nb],
            xap[:, u - xu0, :],
            wts[wi][2][:, wo, n0:n0 + sz],
            start=(u == 0) if started is None else first,
            stop=(u == KT - 1),
        )

    # Startup ladder: interleave the first 2 2/3 token tiles across all 8
    # PSUM banks, with PE stages emitted in the exact order the x/weight
    # pieces arrive on their queues (PSUM accumulation order over u is
    # free).  Tile 2's last block is deferred to the main loop (bank
    # budget).
    for m in range(3, min(5, MT)):
        xts[m] = produce(m)
    nextp = min(5, MT)
    spss = [alloc_ps() for _ in range(2)]
    sps2 = [psx.tile([P, 512], F32, name=f"xps{nb}")[:, :sz]
            for nb, (n0, sz) in enumerate(NBLK[:2])]
    started = set()
    # u4-7 first: the PE's first instruction then gates on BOTH leading
    # weight chunks, after which the stream stays ahead of consumption —
    # one continuous run instead of several stall+p-state-ramp cycles.
    UORDER = [4, 5, 6, 7, 0, 1, 2, 3] + list(range(8, KT))
    for u in UORDER:
        for i in range(2):
            for nb, (n0, sz) in enumerate(NBLK):
                mm(spss[i], xts[i], u, nb, n0, sz, started)
        for nb, (n0, sz) in enumerate(NBLK[:2]):
            mm(sps2, xts[2], u, nb, n0, sz, started)
    for i in range(2):
        xts.pop(i)
        for nb, (n0, sz) in enumerate(NBLK):
            evict(i, nb, n0, sz, spss[i][nb])

    for m in range(2, MT):
        if nextp < MT:
            xts[nextp] = produce(nextp)
            nextp += 1
        xtf = xts.pop(m)
        if m == 2:
            # Finish tile 2: only its last block remains, then evict all.
            nb2, (n20, s2z) = 2, NBLK[2]
            ps2 = psp.tile([P, 512], F32, name="ps2")[:, :s2z]
            for u in range(KT):
                mm([None, None, ps2], xtf, u, nb2, n20, s2z)
            for nb, (n0, sz) in enumerate(NBLK[:2]):
                evict(m, nb, n0, sz, sps2[nb])
            evict(m, nb2, n20, s2z, ps2)
            continue
        pss = alloc_ps()
        if m == MT - 1:
            # Last tile: block-sequential so each block's evict+store
            # overlaps the remaining blocks' matmuls, and stores rotated
            # across queues so the final drain flushes in parallel.
            for nb, (n0, sz) in enumerate(NBLK):
                for u in range(KT):
                    mm(pss, xtf, u, nb, n0, sz)
                evict(m, nb, n0, sz, pss[nb], qi=m + nb)
        else:
            # k-chunk-outer so weight chunks are consumed in arrival order.
            for u in range(KT):
                for nb, (n0, sz) in enumerate(NBLK):
                    mm(pss, xtf, u, nb, n0, sz)
            for nb, (n0, sz) in enumerate(NBLK):
                evict(m, nb, n0, sz, pss[nb])

    ctx.close()


def build_nc(debug=False):
    nc = bacc.Bacc(
        "TRN2",
        target_bir_lowering=False,
        debug=debug,
        num_devices=NCORES,
        enable_asserts=debug,
    )
    xt_d = nc.dram_tensor("xt", [MT, P, KT, P], BF16, kind="ExternalInput").ap()
    wt_d = nc.dram_tensor("wt", [P, KT, OUT_SH], BF16, kind="ExternalInput").ap()
    bb_d = nc.dram_tensor("biasb", [P, OUT_SH], F32, kind="ExternalInput").ap()
    y_d = nc.dram_tensor("y", [T, OUT_SH], F32, kind="ExternalOutput").ap()
    with tile.TileContext(nc) as tc:
        emit_kernel(tc, nc, xt_d, wt_d, bb_d, y_d)
    nc.compile()
    return nc


_NC_CACHE = {}


def _get_nc():
    if "nc" not in _NC_CACHE:
        _NC_CACHE["nc"] = build_nc()
    return _NC_CACHE["nc"]


def prep_inputs(x, qweight, scale, bias):
    """Host-side prep. Returns in_maps for run_bass_kernel_spmd."""
    import ml_dtypes
    x = np.asarray(x)
    qw = np.asarray(qweight)
    sc = np.asarray(scale, dtype=np.float32)
    b = np.asarray(bias, dtype=np.float32)

    # xt[m, p, u, t] = x[128m + t, 128u + p], bf16
    x2 = x.reshape(T, IN).astype(ml_dtypes.bfloat16)
    xt = np.ascontiguousarray(
        x2.reshape(MT, P, KT, P).transpose(0, 3, 2, 1))

    # Dequantize exactly as the reference does (q / scale, f32), then bf16.
    qw2 = qw.reshape(OUT, NG, G)
    w = (qw2.astype(np.float32) / sc.reshape(OUT, NG, 1)).reshape(OUT, IN)
    w_p = np.zeros((OUT_PAD, IN), dtype=ml_dtypes.bfloat16)
    w_p[:OUT] = w.astype(ml_dtypes.bfloat16)
    b_p = np.zeros(OUT_PAD, dtype=np.float32)
    b_p[:OUT] = b

    in_maps = []
    for c in range(NCORES):
        sl = slice(c * OUT_SH, (c + 1) * OUT_SH)
        # wt[p, u, o] = w[o, 128u + p]
        wt = np.ascontiguousarray(
            w_p[sl].reshape(OUT_SH, KT, P).transpose(2, 1, 0))
        in_maps.append({
            "xt": xt,
            "wt": wt,
            "biasb": np.ascontiguousarray(
                np.broadcast_to(b_p[sl][None, :], (P, OUT_SH))
            ),
        })
    return in_maps


def run(x, qweight, scale, bias, trace=False):
    nc = _get_nc()
    in_maps = prep_inputs(x, qweight, scale, bias)
    res = run_bass_kernel_spmd(nc, in_maps, core_ids=list(range(NCORES)),
                               trace=trace)
    ys = [np.asarray(res.results[c]["y"]) for c in range(NCORES)]
    out = np.concatenate(ys, axis=1)[:, :OUT]
    return out.reshape(B, S, OUT).astype(np.float32, copy=False), res


def kernel(x, qweight, scale, bias):
    out, _ = run(x, qweight, scale, bias, trace=False)
    return out
